# revision 3
# baseline (speedup 1.0000x reference)
"""BiRealLinear Trainium2 kernel.

Computes out = binact(x) @ quant_weight(w).T for
  x [4, 2048, 4096] f32, w [4096, 4096] f32  ->  out [4, 2048, 4096] f32

Forward semantics (STE parts drop out in forward):
  binact(x)       = sign(x)                      in {-1, 0, +1}
  quant_weight(w) = mean(|w|, axis=1) * sign(w)  per-output-row scale

So out[t, o] = scale[o] * sum_i sign(x[t,i]) * sign(w[o,i]).

Strategy: 8 cores in a 4 (token) x 2 (out-feature) grid. Each core does a
[2048 x 2048 x 4096] matmul with +/-1 operands held in bf16 (exact), fp32
PSUM accumulation (exact, sums are integers <= 4096), and the fp32 scale
applied at the end. Operands are transposed on device (contraction dim must
sit on SBUF partitions): SWDGE cast-DMA f32->bf16 (DRAM->DRAM, blocked
layout) then xbar transpose-reads into SBUF.
"""

import sys

import numpy as np

try:
    import concourse.bacc as bacc  # noqa: F401
except ImportError:
    sys.path.insert(0, "/opt/trn_rl_repo")

import concourse.bacc as bacc
import concourse.mybir as mybir
import concourse.tile as tile
from concourse.bass_utils import run_bass_kernel_spmd

dt = mybir.dt
AF = mybir.ActivationFunctionType

# ---- problem geometry (hardcoded; full input is [8192, 4096] x [4096, 4096])
B, S, I_FULL, O_FULL = 4, 2048, 4096, 4096
T_FULL = B * S                      # 8192 tokens
T_GRID, O_GRID = 4, 2               # core grid: 4 token shards x 2 out shards
T_SH = T_FULL // T_GRID             # 2048 tokens per core
O_SH = O_FULL // O_GRID             # 2048 out features per core

P = 128                             # partitions
NK = I_FULL // P                    # 32 k-tiles (contraction blocks)
HALVES = 2                          # token halves per core (SBUF accumulator fit)
T_HALF = T_SH // HALVES             # 1024
NT = T_HALF // P                    # 8 t-blocks per half
NPAN = 4                            # o-panels of 512 (one PSUM bank each)
OP = O_SH // NPAN                   # 512
CH = 8                              # k-tiles per PSUM accumulation chunk
NCH = NK // CH                      # 4 chunks

SX_BUFS = 12
SW_BUFS = 12
WA_BUFS = 2
PS_BUFS = 6


def build_nc():
    nc = bacc.Bacc("TRN2", target_bir_lowering=False, debug=False, num_devices=8)
    x = nc.dram_tensor("x", [T_SH, I_FULL], dt.float32, kind="ExternalInput")
    w = nc.dram_tensor("w", [O_SH, I_FULL], dt.float32, kind="ExternalInput")
    out = nc.dram_tensor("out", [T_SH, O_SH], dt.float32, kind="ExternalOutput")

    with tile.TileContext(nc) as tc:
        with (
            tc.tile_pool(name="dram", bufs=1, space="DRAM") as dram,
            tc.tile_pool(name="single", bufs=1) as sb,
            tc.tile_pool(name="sxp", bufs=SX_BUFS) as sxp,
            tc.tile_pool(name="swp", bufs=SW_BUFS) as swp,
            tc.tile_pool(name="wap", bufs=WA_BUFS) as wap,
            tc.tile_pool(name="accp", bufs=NT) as accp,
            tc.tile_pool(name="psp", bufs=PS_BUFS, space="PSUM") as psp,
            tc.tile_pool(name="ps2", bufs=1, space="PSUM") as ps2,
        ):
            # blocked bf16 copies of the operands: [k-block][row][128 cols]
            xb = dram.tile([NK, T_SH, P], dt.bfloat16)
            wb = dram.tile([NK, O_SH, P], dt.bfloat16)

            # constants / persistent tiles
            ones_inv = sb.tile([P, 1], dt.float32)      # 1/I column (k-reduce)
            nc.vector.memset(ones_inv[:], 1.0 / I_FULL)
            ones_bc = sb.tile([1, P], dt.float32)       # 1.0 row (broadcast)
            nc.vector.memset(ones_bc[:], 1.0)
            pabs = sb.tile([P, O_SH], dt.float32)       # per-lane |w| partials
            nc.vector.memset(pabs[:], 0.0)
            srow = sb.tile([1, O_SH], dt.float32)       # scale row
            scale_bc = sb.tile([P, O_SH], dt.float32)   # scale broadcast to 128p

            # phase 0: DRAM->DRAM cast-copies (SWDGE), k-block granular so the
            # transpose-reads can start as soon as their block lands.
            for b in range(NK):
                nc.gpsimd.dma_start(xb[:][b, :, :], x[:, b * P:(b + 1) * P])
                nc.gpsimd.dma_start(wb[:][b, :, :], w[:, b * P:(b + 1) * P])

            for h in range(HALVES):
                # operand load: transpose-reads + sign (and |w| partials, h==0)
                sx_t = []
                sw_t = []
                for gk in range(NK):
                    sx = sxp.tile([P, T_HALF], dt.bfloat16, tag="sx", name=f"sx_{h}_{gk}")
                    nc.sync.dma_start(
                        sx[:], xb[:][gk, h * T_HALF:(h + 1) * T_HALF, :],
                        transpose=True,
                    )
                    nc.scalar.sign(sx[:], sx[:])
                    sx_t.append(sx)

                    sw = swp.tile([P, O_SH], dt.bfloat16, tag="sw", name=f"sw_{h}_{gk}")
                    nc.sync.dma_start(sw[:], wb[:][gk, :, :], transpose=True)
                    if h == 0:
                        wa = wap.tile([P, O_SH], dt.float32, tag="wa", name=f"wa_{gk}")
                        nc.scalar.activation(wa[:], sw[:], AF.Abs)
                        nc.vector.tensor_add(pabs[:], pabs[:], wa[:])
                    nc.scalar.sign(sw[:], sw[:])
                    sw_t.append(sw)

                # chunked matmuls with PSUM accumulation, drained into SBUF f32
                accs = []
                for t in range(NT):
                    accs.append(accp.tile([P, O_SH], dt.float32, tag="acc", name=f"acc_{h}_{t}"))
                for c in range(NCH):
                    for t in range(NT):
                        pst = [
                            psp.tile([P, OP], dt.float32, tag="ps",
                                     name=f"ps_{h}_{c}_{t}_{pp}")
                            for pp in range(NPAN)
                        ]
                        for kk in range(CH):
                            gk = c * CH + kk
                            lhsT = sx_t[gk][:, t * P:(t + 1) * P]
                            for p in range(NPAN):
                                nc.tensor.matmul(
                                    pst[p][:],
                                    lhsT=lhsT,
                                    rhs=sw_t[gk][:, p * OP:(p + 1) * OP],
                                    start=(kk == 0),
                                    stop=(kk == CH - 1),
                                )
                        for p in range(NPAN):
                            asl = accs[t][:, p * OP:(p + 1) * OP]
                            if c == 0:
                                nc.vector.tensor_copy(asl, pst[p][:])
                            else:
                                nc.vector.tensor_add(asl, asl, pst[p][:])

                if h == 0:
                    # scale[o] = (1/I) * sum_p pabs[p, o]; then broadcast to 128
                    # partitions via a K=1 matmul with a ones column.
                    for p in range(NPAN):
                        pss = ps2.tile([1, OP], dt.float32, tag="pss", name=f"pss_{p}")
                        nc.tensor.matmul(
                            pss[:], lhsT=ones_inv[:],
                            rhs=pabs[:, p * OP:(p + 1) * OP],
                            start=True, stop=True,
                        )
                        nc.scalar.copy(srow[:, p * OP:(p + 1) * OP], pss[:])
                    for p in range(NPAN):
                        psb = ps2.tile([P, OP], dt.float32, tag="psb", name=f"psb_{p}")
                        nc.tensor.matmul(
                            psb[:], lhsT=ones_bc[:],
                            rhs=srow[:, p * OP:(p + 1) * OP],
                            start=True, stop=True,
                        )
                        nc.vector.tensor_copy(scale_bc[:, p * OP:(p + 1) * OP], psb[:])

                for t in range(NT):
                    nc.vector.tensor_mul(accs[t][:], accs[t][:], scale_bc[:])
                    row0 = (h * NT + t) * P
                    nc.sync.dma_start(out[row0:row0 + P, :], accs[t][:])

    nc.compile()
    return nc


_NC_CACHE = None


def _get_nc():
    global _NC_CACHE
    if _NC_CACHE is None:
        _NC_CACHE = build_nc()
    return _NC_CACHE


def kernel(x, weight):
    x = np.ascontiguousarray(np.asarray(x, dtype=np.float32))
    weight = np.ascontiguousarray(np.asarray(weight, dtype=np.float32))
    xr = x.reshape(T_FULL, I_FULL)

    nc = _get_nc()
    in_maps = []
    for core in range(8):
        ti, oj = core // O_GRID, core % O_GRID
        in_maps.append({
            "x": np.ascontiguousarray(xr[ti * T_SH:(ti + 1) * T_SH]),
            "w": np.ascontiguousarray(weight[oj * O_SH:(oj + 1) * O_SH]),
        })
    res = run_bass_kernel_spmd(nc, in_maps, list(range(8)))
    out = np.empty((T_FULL, O_FULL), dtype=np.float32)
    for core in range(8):
        ti, oj = core // O_GRID, core % O_GRID
        out[ti * T_SH:(ti + 1) * T_SH, oj * O_SH:(oj + 1) * O_SH] = (
            res.results[core]["out"]
        )
    return out.reshape(B, S, O_FULL)


# revision 5
# speedup vs baseline: 1.1765x; 1.1765x over previous
"""BiRealLinear Trainium2 kernel.

Computes out = binact(x) @ quant_weight(w).T for
  x [4, 2048, 4096] f32, w [4096, 4096] f32  ->  out [4, 2048, 4096] f32

Forward semantics (STE parts drop out in forward):
  binact(x)       = sign(x)                      in {-1, 0, +1}
  quant_weight(w) = mean(|w|, axis=1) * sign(w)  per-output-row scale

So out[t, o] = scale[o] * sum_i sign(x[t,i]) * sign(w[o,i]).

Strategy: 8 cores in a 4 (token) x 2 (out-feature) grid. Each core does a
[2048 x 2048 x 4096] matmul with +/-1 operands held in bf16 (exact), fp32
PSUM accumulation (exact, sums are integers <= 4096), and the fp32 scale
applied at the end. Operands are transposed on device (contraction dim must
sit on SBUF partitions): SWDGE cast-DMA f32->bf16 (DRAM->DRAM, contiguous
column slabs for big descriptors) then xbar transpose-reads into SBUF,
alternating the two HWDGE rings (sync/scalar).
"""

import sys

import numpy as np

try:
    import concourse.bacc as bacc  # noqa: F401
except ImportError:
    sys.path.insert(0, "/opt/trn_rl_repo")

import concourse.bacc as bacc
import concourse.mybir as mybir
import concourse.tile as tile
from concourse.bass_utils import run_bass_kernel_spmd

dt = mybir.dt
AF = mybir.ActivationFunctionType

# ---- problem geometry (hardcoded; full input is [8192, 4096] x [4096, 4096])
B, S, I_FULL, O_FULL = 4, 2048, 4096, 4096
T_FULL = B * S                      # 8192 tokens
T_GRID, O_GRID = 4, 2               # core grid: 4 token shards x 2 out shards
T_SH = T_FULL // T_GRID             # 2048 tokens per core
O_SH = O_FULL // O_GRID             # 2048 out features per core

P = 128                             # partitions
NK = I_FULL // P                    # 32 k-tiles (contraction blocks)
HALVES = 2                          # token halves per core (SBUF accumulator fit)
T_HALF = T_SH // HALVES             # 1024
NT = T_HALF // P                    # 8 t-blocks per half
NPAN = 4                            # o-panels of 512 (one PSUM bank each)
OP = O_SH // NPAN                   # 512
CH = 8                              # k-tiles per PSUM accumulation chunk
NCH = NK // CH                      # 4 chunks
CSLAB = CH * P                      # 1024 source cols per cast slab

SX_BUFS = 12
SW_BUFS = 12
WA_BUFS = 2
PS_BUFS = 3                         # [128, 1024] psum tiles (2 banks each)


def build_nc():
    nc = bacc.Bacc("TRN2", target_bir_lowering=False, debug=False, num_devices=8)
    x = nc.dram_tensor("x", [T_SH, I_FULL], dt.float32, kind="ExternalInput")
    w = nc.dram_tensor("w", [O_SH, I_FULL], dt.float32, kind="ExternalInput")
    out = nc.dram_tensor("out", [T_SH, O_SH], dt.float32, kind="ExternalOutput")

    with tile.TileContext(nc) as tc:
        with (
            tc.tile_pool(name="dram", bufs=1, space="DRAM") as dram,
            tc.tile_pool(name="single", bufs=1) as sb,
            tc.tile_pool(name="sxp", bufs=SX_BUFS) as sxp,
            tc.tile_pool(name="swp", bufs=SW_BUFS) as swp,
            tc.tile_pool(name="wap", bufs=WA_BUFS) as wap,
            tc.tile_pool(name="accp", bufs=NT) as accp,
            tc.tile_pool(name="psp", bufs=PS_BUFS, space="PSUM") as psp,
        ):
            # flat bf16 copies of the operands (same [rows, I] layout)
            xb = dram.tile([T_SH, I_FULL], dt.bfloat16)
            wb = dram.tile([O_SH, I_FULL], dt.bfloat16)

            # constants / persistent tiles
            ones_inv = sb.tile([P, 1], dt.float32)      # 1/I column (k-reduce)
            nc.vector.memset(ones_inv[:], 1.0 / I_FULL)
            ones_bc = sb.tile([1, P], dt.float32)       # 1.0 row (broadcast)
            nc.vector.memset(ones_bc[:], 1.0)
            pabs = sb.tile([P, O_SH], dt.float32)       # per-lane |w| partials
            nc.vector.memset(pabs[:], 0.0)
            srow = sb.tile([1, O_SH], dt.float32)       # scale row
            scale_bc = sb.tile([P, O_SH], dt.float32)   # scale broadcast to 128p

            # phase 0: DRAM->DRAM cast-copies (SWDGE) in contiguous column
            # slabs (4KB row chunks -> big descriptors), ordered so chunk c of
            # half 0 unblocks as early as possible.
            for c in range(NCH):
                c0 = c * CSLAB
                nc.gpsimd.dma_start(wb[:][:, c0:c0 + CSLAB], w[:, c0:c0 + CSLAB])
                nc.gpsimd.dma_start(
                    xb[:][0:T_HALF, c0:c0 + CSLAB], x[0:T_HALF, c0:c0 + CSLAB])
            for c in range(NCH):
                c0 = c * CSLAB
                nc.gpsimd.dma_start(
                    xb[:][T_HALF:T_SH, c0:c0 + CSLAB],
                    x[T_HALF:T_SH, c0:c0 + CSLAB])

            for h in range(HALVES):
                # operand load: transpose-reads + sign (and |w| partials, h==0)
                sx_t = []
                sw_t = []
                for gk in range(NK):
                    teng = nc.sync
                    sx = sxp.tile([P, T_HALF], dt.bfloat16, tag="sx",
                                  name=f"sx_{h}_{gk}")
                    teng.dma_start(
                        sx[:], xb[:][h * T_HALF:(h + 1) * T_HALF,
                                     gk * P:(gk + 1) * P],
                        transpose=True,
                    )
                    nc.scalar.sign(sx[:], sx[:])
                    sx_t.append(sx)

                    sw = swp.tile([P, O_SH], dt.bfloat16, tag="sw",
                                  name=f"sw_{h}_{gk}")
                    nc.sync.dma_start(
                        sw[:], wb[:][:, gk * P:(gk + 1) * P], transpose=True)
                    if h == 0:
                        wa = wap.tile([P, O_SH], dt.float32, tag="wa",
                                      name=f"wa_{gk}")
                        nc.scalar.activation(wa[:], sw[:], AF.Abs)
                        nc.vector.tensor_add(pabs[:], pabs[:], wa[:])
                    nc.scalar.sign(sw[:], sw[:])
                    sw_t.append(sw)

                # chunked matmuls with PSUM accumulation, drained into SBUF f32
                accs = []
                for t in range(NT):
                    accs.append(accp.tile([P, O_SH], dt.float32, tag="acc",
                                          name=f"acc_{h}_{t}"))
                for c in range(NCH):
                    for t in range(NT):
                        pst = [
                            psp.tile([P, 2 * OP], dt.float32, tag="ps",
                                     name=f"ps_{h}_{c}_{t}_{j}")
                            for j in range(2)
                        ]
                        for kk in range(CH):
                            gk = c * CH + kk
                            lhsT = sx_t[gk][:, t * P:(t + 1) * P]
                            for p in range(NPAN):
                                nc.tensor.matmul(
                                    pst[p // 2][:, (p % 2) * OP:(p % 2 + 1) * OP],
                                    lhsT=lhsT,
                                    rhs=sw_t[gk][:, p * OP:(p + 1) * OP],
                                    start=(kk == 0),
                                    stop=(kk == CH - 1),
                                )
                        for j in range(2):
                            asl = accs[t][:, j * 2 * OP:(j + 1) * 2 * OP]
                            if c == 0:
                                nc.vector.tensor_copy(asl, pst[j][:])
                            else:
                                nc.vector.tensor_add(asl, asl, pst[j][:])

                if h == 0:
                    # scale[o] = (1/I) * sum_p pabs[p, o]; then broadcast to
                    # 128 partitions via a K=1 matmul with a ones column.
                    for j in range(2):
                        pss = psp.tile([P, 2 * OP], dt.float32, tag="ps",
                                       name=f"pss_{j}")
                        for p in range(2):
                            nc.tensor.matmul(
                                pss[0:1, p * OP:(p + 1) * OP],
                                lhsT=ones_inv[:],
                                rhs=pabs[:, (2 * j + p) * OP:(2 * j + p + 1) * OP],
                                start=True, stop=True,
                            )
                        nc.scalar.copy(
                            srow[:, j * 2 * OP:(j + 1) * 2 * OP], pss[0:1, :])
                    for j in range(2):
                        psb = psp.tile([P, 2 * OP], dt.float32, tag="ps",
                                       name=f"psb_{j}")
                        for p in range(2):
                            nc.tensor.matmul(
                                psb[:, p * OP:(p + 1) * OP],
                                lhsT=ones_bc[:],
                                rhs=srow[0:1,
                                         (2 * j + p) * OP:(2 * j + p + 1) * OP],
                                start=True, stop=True,
                            )
                        nc.vector.tensor_copy(
                            scale_bc[:, j * 2 * OP:(j + 1) * 2 * OP], psb[:])

                for t in range(NT):
                    nc.vector.tensor_mul(accs[t][:], accs[t][:], scale_bc[:])
                    row0 = (h * NT + t) * P
                    nc.scalar.dma_start(out[row0:row0 + P, :], accs[t][:])

    nc.compile()
    return nc


_NC_CACHE = None


def _get_nc():
    global _NC_CACHE
    if _NC_CACHE is None:
        _NC_CACHE = build_nc()
    return _NC_CACHE


def kernel(x, weight):
    x = np.ascontiguousarray(np.asarray(x, dtype=np.float32))
    weight = np.ascontiguousarray(np.asarray(weight, dtype=np.float32))
    xr = x.reshape(T_FULL, I_FULL)

    nc = _get_nc()
    in_maps = []
    for core in range(8):
        ti, oj = core // O_GRID, core % O_GRID
        in_maps.append({
            "x": np.ascontiguousarray(xr[ti * T_SH:(ti + 1) * T_SH]),
            "w": np.ascontiguousarray(weight[oj * O_SH:(oj + 1) * O_SH]),
        })
    res = run_bass_kernel_spmd(nc, in_maps, list(range(8)))
    out = np.empty((T_FULL, O_FULL), dtype=np.float32)
    for core in range(8):
        ti, oj = core // O_GRID, core % O_GRID
        out[ti * T_SH:(ti + 1) * T_SH, oj * O_SH:(oj + 1) * O_SH] = (
            res.results[core]["out"]
        )
    return out.reshape(B, S, O_FULL)
